# revision 43
# baseline (speedup 1.0000x reference)
"""2-layer LSTM (B=1024, T=256, I=64, H=128) + FC head on 8 NeuronCores.

Data-parallel: batch sharded 8 ways (128 rows/core), weights replicated.
On-chip orientation keeps state transposed (hT: [H partitions, B free]) so the
recurrence matmuls, activations and cell updates never need a transpose.

The kernel is latency-bound on the per-step serial chain
  h-proj matmuls -> sigmoid -> cell update (DVE) -> tanh -> h (DVE)
with layer 0 (step t) and layer 1 (step t-1) as two interleaved chains
hiding each other's latency. Key choices:
 - fp16 matmul operands and sigmoid outputs (PE 4x vs fp32; DVE TT 2x),
   cell state kept at half scale (c' = c/2) so the tanh is tanh(2c') via
   the activation scale and the cell update is pure fp16 tensor-tensor.
 - One PSUM bank per (layer, gate): each bank's accumulation groups stay
   strictly sequential (x start -> h stop -> sigma read -> next x), which
   lets the x-projections issue a step early while only the 4 h-proj
   matmuls sit on the critical path.
 - Gate order (i, f, g, o): sigmoid over i,f,g fires after 3 of the 4
   h-proj matmuls; the o-gate sigmoid (only needed by h', after tanh)
   runs off-path. The g-gate's tanh is 2*sigmoid(2z)-1 with the 2z
   folded into the weights.
"""

import numpy as np

B, T, I, H = 1024, 256, 64, 128
NCORES = 8
BC = B // NCORES  # 128 batch rows per core
XCHUNK = 32  # timesteps per staged x DMA chunk
CH = 2  # timesteps per x-projection chunk (double-buffered inside psum banks)
HALF = "float16"  # matmul operands + sigmoid outputs; cell state stays fp32


def _mm_np_dtype():
    if HALF == "bfloat16":
        import ml_dtypes

        return ml_dtypes.bfloat16
    if HALF == "float16":
        return np.float16
    return np.float32


_cache = {}


def _build(has_b1, has_bfc, nsteps):
    import concourse.bacc as bacc
    import concourse.tile as tile
    import concourse.mybir as mybir

    f32 = mybir.dt.float32
    mdt = getattr(mybir.dt, HALF)
    Alu = mybir.AluOpType
    Act = mybir.ActivationFunctionType

    nc = bacc.Bacc("TRN2", target_bir_lowering=False, debug=False)

    xt_d = nc.dram_tensor("xt", [I + 2, nsteps, BC], mdt, kind="ExternalInput")
    w0x_d = nc.dram_tensor("w0x", [I + 2, 4 * H], mdt, kind="ExternalInput")
    w0h_d = nc.dram_tensor("w0h", [H, 4 * H], mdt, kind="ExternalInput")
    w1x_d = nc.dram_tensor("w1x", [H, 4 * H], mdt, kind="ExternalInput")
    w1h_d = nc.dram_tensor("w1h", [H, 4 * H], mdt, kind="ExternalInput")
    wfc_d = nc.dram_tensor("wfc", [H, 1], mdt, kind="ExternalInput")
    b1_d = nc.dram_tensor("b1", [4, 1, H], mdt, kind="ExternalInput") if has_b1 else None
    bfc_d = nc.dram_tensor("bfc", [1, 1], mdt, kind="ExternalInput") if has_bfc else None
    out_d = nc.dram_tensor("out", [1, BC], f32, kind="ExternalOutput")

    with tile.TileContext(nc) as tc:
        with (
            tc.tile_pool(name="singles", bufs=1) as singles,
            tc.tile_pool(name="sg", bufs=3) as sgp,
            tc.tile_pool(name="tmp", bufs=4) as tmpp,
            tc.tile_pool(name="ps", bufs=1, space="PSUM") as psp,
        ):
            # DMA issue order matters for startup: the first matmuls need
            # w0x/w0h + x chunk 0, so those go first; bulk x chunks last.
            def load_wgroup(dram, k, tag):
                w = singles.tile([k, 4 * H], mdt, tag=tag, name=tag)
                nc.sync.dma_start(out=w[:], in_=dram.ap())
                return [w[:, q * H : (q + 1) * H] for q in range(4)]

            w0x = load_wgroup(w0x_d, I + 2, "w0x")

            # x chunks: a tiny first chunk (FCH steps) lands fast so the
            # first x-projection starts as early as possible
            FCH = min(4, nsteps)
            xta = xt_d.ap()
            starts = [0, FCH] if nsteps > FCH else [0]
            while starts[-1] + XCHUNK < nsteps:
                starts.append(starts[-1] + XCHUNK)
            bounds = list(zip(starts, starts[1:] + [nsteps]))
            xts = [None] * len(bounds)

            def load_xchunk(j):
                t0, t1 = bounds[j]
                xt_t = singles.tile([I + 2, (t1 - t0) * BC], mdt, tag=f"xt{j}", name=f"xt{j}")
                nc.sync.dma_start(
                    out=xt_t[:], in_=xta[:, t0:t1, :].rearrange("p t b -> p (t b)")
                )
                xts[j] = xt_t

            load_xchunk(0)
            w0h = load_wgroup(w0h_d, H, "w0h")
            if len(bounds) > 1:
                load_xchunk(1)
            w1x = load_wgroup(w1x_d, H, "w1x")
            w1h = load_wgroup(w1h_d, H, "w1h")
            wfc = singles.tile([H, 1], mdt, tag="wfc", name="wfc")
            nc.sync.dma_start(out=wfc[:], in_=wfc_d.ap())
            for j in range(2, len(bounds)):
                load_xchunk(j)

            def load_w(dram, k, q, tag):
                w = singles.tile([k, H], mdt, tag=f"{tag}{q}", name=f"{tag}{q}")
                nc.sync.dma_start(out=w[:], in_=dram.ap()[q])
                return w
            b1 = None
            ones = None
            if has_b1 or has_bfc:
                ones = singles.tile([1, BC], mdt, tag="ones", name="ones")
                nc.vector.memset(ones[:], 1.0)
            if has_b1:
                b1 = [load_w(b1_d, 1, q, "b1") for q in range(4)]
            bfc = None
            if has_bfc:
                bfc = singles.tile([1, 1], mdt, tag="bfc", name="bfc")
                nc.sync.dma_start(out=bfc[:], in_=bfc_d.ap())

            # half-scale cell state: cs holds c/2 (fp16 so DVE 2x applies)
            cs = []
            for layer in range(2):
                c = singles.tile([H, BC], mdt, tag=f"c{layer}", name=f"c{layer}")
                nc.vector.memset(c[:], 0.0)
                cs.append(c)
            RING = 4
            rings = [
                singles.tile([H, RING * BC], mdt, tag=f"h{layer}", name=f"h{layer}")
                for layer in range(2)
            ]

            def hslot(layer, t):
                s = t % RING
                return rings[layer][:, s * BC : (s + 1) * BC]

            # PSUM layout: one full 2KB bank per (layer, gate); only the first
            # BC columns are used. No step rotation: step t+1's x-projection
            # (start=True, lazily zeroing the bank) is WAR-serialized after
            # sigma(t)'s read of the same bytes, so each bank's accumulation
            # groups stay strictly sequential: x(start) -> h(stop) -> sigma
            # read -> x(start) -> ...
            # Bank order: L0 i,f,g = 0-2; L1 i,f,g = 3-5; o-gates last (L0=6,
            # L1=7). Sigmoid over each layer's i,f,g fires after only 3 of 4
            # h-proj matmuls; the o-gate sigmoids run off the critical path.
            psu = psp.tile([H, 8, 512], f32, tag="ps", name="ps")

            def gate_out(layer, q):
                bank = layer * 3 + q if q < 3 else 6 + layer
                return psu[:, bank : bank + 1, 0:BC].rearrange("p a b -> p (a b)")

            def x_rhs(t):
                if t < FCH:
                    j, r = 0, t
                else:
                    j = 1 + (t - FCH) // XCHUNK
                    r = (t - FCH) % XCHUNK
                return xts[j][:, r * BC : (r + 1) * BC]

            def emit_l0_x(t):
                for q in range(4):
                    nc.tensor.matmul(
                        gate_out(0, q), w0x[q], x_rhs(t),
                        start=True, stop=(t == 0),
                    )

            def emit_l0_h(t):
                for q in range(4):
                    nc.tensor.matmul(
                        gate_out(0, q), w0h[q], hslot(0, t - 1),
                        start=False, stop=True,
                    )

            def emit_l1_x(t):
                for q in range(4):
                    nc.tensor.matmul(
                        gate_out(1, q), w1x[q], hslot(0, t),
                        start=True, stop=(t == 0 and not has_b1),
                    )
                    if has_b1:
                        nc.tensor.matmul(
                            gate_out(1, q), b1[q][:], ones[:],
                            start=False, stop=(t == 0),
                        )

            def emit_l1_h(t):
                for q in range(4):
                    nc.tensor.matmul(
                        gate_out(1, q), w1h[q], hslot(1, t - 1),
                        start=False, stop=True,
                    )

            def emit_sig_o(layers):
                # sigmoid over o-gate bank(s); off the critical path (only
                # h' needs it, after tanh)
                sgo = sgp.tile([H, 2 * BC], mdt, tag="sgo", name="sgo")
                lo, hi = 6 + min(layers), 7 + max(layers)
                nc.scalar.activation(
                    sgo[:, (lo - 6) * BC : (hi - 6) * BC].rearrange(
                        "p (g b) -> p g b", g=hi - lo
                    ),
                    psu[:, lo:hi, 0:BC], Act.Sigmoid,
                )
                return sgo

            def act_dve(layer, t):
                # gate order in banks/sg: (i, f, g, o). Sigmoid over i,f,g
                # fires after only 3 of 4 h-proj matmuls; the o-gate sigmoid
                # is needed only by h' (after tanh), so it runs off-path.
                sg = sgp.tile([H, 3 * BC], mdt, tag=f"sg{layer}", name=f"sg{layer}")
                nc.scalar.activation(
                    sg[:].rearrange("p (g b) -> p g b", g=3),
                    psu[:, layer * 3 : layer * 3 + 3, 0:BC], Act.Sigmoid,
                )
                t1_ = tmpp.tile([H, BC], mdt, tag=f"t1_{layer}", name=f"t1_{layer}")
                # (sig_g - 0.5) * sig_i  == 0.5 * i * tanh(g_pre)
                nc.vector.scalar_tensor_tensor(
                    t1_[:], sg[:, 2 * BC : 3 * BC], 0.5, sg[:, 0:BC],
                    Alu.subtract, Alu.mult,
                )
                t2_ = tmpp.tile([H, BC], mdt, tag=f"t2_{layer}", name=f"t2_{layer}")
                nc.vector.tensor_mul(t2_[:], sg[:, BC : 2 * BC], cs[layer][:])
                # c' = t1 + t2 = (i*tanh(g_pre) + f*c)/2  (half-scale state)
                nc.vector.tensor_add(cs[layer][:], t1_[:], t2_[:])
                sgo = emit_sig_o((layer,))
                th = tmpp.tile([H, BC], mdt, tag=f"th{layer}", name=f"th{layer}")
                # tanh(c) = tanh(2*c')
                nc.scalar.activation(th[:], cs[layer][:], Act.Tanh, scale=2.0)
                nc.vector.tensor_mul(
                    hslot(layer, t), sgo[:, layer * BC : (layer + 1) * BC], th[:]
                )

            emit_l0_x(0)
            for t in range(nsteps):
                if t >= 1:
                    emit_l0_h(t)
                    emit_l1_x(t - 1)
                    if t - 1 >= 1:
                        emit_l1_h(t - 1)
                act_dve(0, t)
                if t >= 1:
                    act_dve(1, t - 1)
                # next step's x-projections, emitted after sigma(t)'s read so
                # the bank WAR serializes them behind it (off critical path)
                if t + 1 < nsteps:
                    emit_l0_x(t + 1)
            emit_l1_x(nsteps - 1)
            if nsteps - 1 >= 1:
                emit_l1_h(nsteps - 1)
            act_dve(1, nsteps - 1)

            # FC head reuses a closed L0 psum bank region.
            pf = psu[0:1, 0:1, 0:BC].rearrange("p a b -> p (a b)")
            nc.tensor.matmul(
                pf, wfc[:], hslot(1, nsteps - 1),
                start=True, stop=not has_bfc,
            )
            if has_bfc:
                nc.tensor.matmul(pf, bfc[:], ones[:], start=False, stop=True)
            ot = singles.tile([1, BC], f32, tag="ot", name="ot")
            nc.vector.tensor_copy(ot[:], pf)
            nc.sync.dma_start(out=out_d.ap(), in_=ot[:])

    nc.compile()
    return nc


def _prep_weights(Wih, Whh, b, in_dim, fold_bias):
    """Repack [4H, in] PyTorch-gate-order (i,f,g,o) weights into per-gate
    lhsT tiles [in(+1), H], g scaled by 2 (tanh(g) = 2*sigmoid(2g) - 1)."""
    order = [0, 1, 2, 3]  # i, f, g, o
    pad = 2 if fold_bias else 0
    wx = np.zeros((4, in_dim + pad, H), np.float32)
    wh = np.empty((4, H, H), np.float32)
    for qi, q in enumerate(order):
        scale = 2.0 if q == 2 else 1.0
        wx[qi, :in_dim] = (Wih[q * H : (q + 1) * H] * scale).T
        if fold_bias:
            wx[qi, in_dim] = b[q * H : (q + 1) * H] * scale
        wh[qi] = (Whh[q * H : (q + 1) * H] * scale).T
    return wx, wh


def kernel(x, Wih0, Whh0, b0, Wih1, Whh1, b1, Wfc, bfc, _nsteps=T):
    from concourse.bass_utils import run_bass_kernel_spmd

    x = np.asarray(x, np.float32)
    nsteps = _nsteps
    has_b1 = bool(np.any(np.asarray(b1)))
    has_bfc = bool(np.any(np.asarray(bfc)))

    w0x, w0h = _prep_weights(np.asarray(Wih0, np.float32), np.asarray(Whh0, np.float32),
                             np.asarray(b0, np.float32), I, True)
    w1x, w1h = _prep_weights(np.asarray(Wih1, np.float32), np.asarray(Whh1, np.float32),
                             np.asarray(b1, np.float32), H, False)
    # [4, K, H] -> [K, 4H] so each weight tensor lands in SBUF as one DMA
    w0x, w0h, w1x, w1h = (
        np.ascontiguousarray(w.transpose(1, 0, 2).reshape(w.shape[1], 4 * H))
        for w in (w0x, w0h, w1x, w1h)
    )
    wfc = np.ascontiguousarray(np.asarray(Wfc, np.float32).reshape(1, H).T)

    key = (has_b1, has_bfc, nsteps)
    if key not in _cache:
        _cache[key] = _build(has_b1, has_bfc, nsteps)
    nc = _cache[key]

    mnp = _mm_np_dtype()
    in_maps = []
    for c in range(NCORES):
        xc = x[c * BC : (c + 1) * BC, :nsteps]  # [BC, t, I]
        xt = np.zeros((I + 2, nsteps, BC), np.float32)
        xt[:I] = xc.transpose(2, 1, 0)
        xt[I] = 1.0
        m = {"xt": xt.astype(mnp), "w0x": w0x.astype(mnp), "w0h": w0h.astype(mnp),
             "w1x": w1x.astype(mnp), "w1h": w1h.astype(mnp), "wfc": wfc.astype(mnp)}
        if has_b1:
            border = [0, 1, 2, 3]
            bb = np.empty((4, 1, H), np.float32)
            for qi, q in enumerate(border):
                bb[qi, 0] = np.asarray(b1, np.float32)[q * H : (q + 1) * H] * (2.0 if q == 2 else 1.0)
            m["b1"] = bb.astype(mnp)
        if has_bfc:
            m["bfc"] = np.asarray(bfc, np.float32).reshape(1, 1).astype(mnp)
        in_maps.append(m)

    res = run_bass_kernel_spmd(nc, in_maps, list(range(NCORES)))
    globals()["LAST_RESULT"] = res
    globals()["LAST_RUN"] = (nc, in_maps)
    out = np.empty((B, 1), np.float32)
    for c in range(NCORES):
        out[c * BC : (c + 1) * BC, 0] = res.results[c]["out"][0]
    return out


def bench(iters=6):
    """Re-run the last compiled kernel, returning per-call wall seconds."""
    import time
    from concourse.bass_utils import run_bass_kernel_spmd

    nc, in_maps = globals()["LAST_RUN"]
    times = []
    for _ in range(iters):
        t0 = time.perf_counter()
        run_bass_kernel_spmd(nc, in_maps, list(range(NCORES)))
        times.append(time.perf_counter() - t0)
    return times



# revision 44
# speedup vs baseline: 1.1937x; 1.1937x over previous
"""2-layer LSTM (B=1024, T=256, I=64, H=128) + FC head on 8 NeuronCores.

Data-parallel: batch sharded 8 ways (128 rows/core), weights replicated.
On-chip orientation keeps state transposed (hT: [H partitions, B free]) so the
recurrence matmuls, activations and cell updates never need a transpose.

The kernel is latency-bound on the per-step serial chain
  h-proj matmuls -> sigmoid -> cell update (DVE) -> tanh -> h (DVE)
with layer 0 (step t) and layer 1 (step t-1) as two interleaved chains
hiding each other's latency. Key choices:
 - fp16 matmul operands and sigmoid outputs (PE 4x vs fp32; DVE TT 2x),
   cell state kept at half scale (c' = c/2) so the tanh is tanh(2c') via
   the activation scale and the cell update is pure fp16 tensor-tensor.
 - One PSUM bank per (layer, gate): each bank's accumulation groups stay
   strictly sequential (x start -> h stop -> sigma read -> next x), which
   lets the x-projections issue a step early while only the 4 h-proj
   matmuls sit on the critical path.
 - Gate order (i, f, g, o): sigmoid over i,f,g fires after 3 of the 4
   h-proj matmuls; the o-gate sigmoid (only needed by h', after tanh)
   runs off-path. The g-gate's tanh is 2*sigmoid(2z)-1 with the 2z
   folded into the weights.
"""

import numpy as np

B, T, I, H = 1024, 256, 64, 128
NCORES = 8
BC = B // NCORES  # 128 batch rows per core
XCHUNK = 32  # timesteps per staged x DMA chunk
CH = 2  # timesteps per x-projection chunk (double-buffered inside psum banks)
HALF = "float16"  # matmul operands + sigmoid outputs; cell state stays fp32


def _mm_np_dtype():
    if HALF == "bfloat16":
        import ml_dtypes

        return ml_dtypes.bfloat16
    if HALF == "float16":
        return np.float16
    return np.float32


_cache = {}


def _build(has_b1, has_bfc, nsteps):
    import concourse.bacc as bacc
    import concourse.tile as tile
    import concourse.mybir as mybir

    f32 = mybir.dt.float32
    mdt = getattr(mybir.dt, HALF)
    Alu = mybir.AluOpType
    Act = mybir.ActivationFunctionType

    nc = bacc.Bacc("TRN2", target_bir_lowering=False, debug=False)

    xt_d = nc.dram_tensor("xt", [I + 2, nsteps, BC], mdt, kind="ExternalInput")
    w0x_d = nc.dram_tensor("w0x", [I + 2, 4 * H], mdt, kind="ExternalInput")
    w0h_d = nc.dram_tensor("w0h", [H, 4 * H], mdt, kind="ExternalInput")
    w1x_d = nc.dram_tensor("w1x", [H, 4 * H], mdt, kind="ExternalInput")
    w1h_d = nc.dram_tensor("w1h", [H, 4 * H], mdt, kind="ExternalInput")
    wfc_d = nc.dram_tensor("wfc", [H, 1], mdt, kind="ExternalInput")
    b1_d = nc.dram_tensor("b1", [4, 1, H], mdt, kind="ExternalInput") if has_b1 else None
    bfc_d = nc.dram_tensor("bfc", [1, 1], mdt, kind="ExternalInput") if has_bfc else None
    out_d = nc.dram_tensor("out", [1, BC], f32, kind="ExternalOutput")

    with tile.TileContext(nc) as tc:
        with (
            tc.tile_pool(name="singles", bufs=1) as singles,
            tc.tile_pool(name="sg", bufs=3) as sgp,
            tc.tile_pool(name="tmp", bufs=4) as tmpp,
            tc.tile_pool(name="ps", bufs=1, space="PSUM") as psp,
        ):
            # DMA issue order matters for startup: the first matmuls need
            # w0x/w0h + x chunk 0, so those go first; bulk x chunks last.
            def load_wgroup(dram, k, tag):
                w = singles.tile([k, 4 * H], mdt, tag=tag, name=tag)
                nc.sync.dma_start(out=w[:], in_=dram.ap())
                return [w[:, q * H : (q + 1) * H] for q in range(4)]

            w0x = load_wgroup(w0x_d, I + 2, "w0x")
            w0h = load_wgroup(w0h_d, H, "w0h")

            xta = xt_d.ap()
            nchunk = (nsteps + XCHUNK - 1) // XCHUNK
            xts = [None] * nchunk

            def load_xchunk(j):
                t0, t1 = j * XCHUNK, min((j + 1) * XCHUNK, nsteps)
                xt_t = singles.tile([I + 2, (t1 - t0) * BC], mdt, tag=f"xt{j}", name=f"xt{j}")
                nc.sync.dma_start(
                    out=xt_t[:], in_=xta[:, t0:t1, :].rearrange("p t b -> p (t b)")
                )
                xts[j] = xt_t

            load_xchunk(0)
            w1x = load_wgroup(w1x_d, H, "w1x")
            w1h = load_wgroup(w1h_d, H, "w1h")
            wfc = singles.tile([H, 1], mdt, tag="wfc", name="wfc")
            nc.sync.dma_start(out=wfc[:], in_=wfc_d.ap())
            for j in range(1, nchunk):
                load_xchunk(j)

            def load_w(dram, k, q, tag):
                w = singles.tile([k, H], mdt, tag=f"{tag}{q}", name=f"{tag}{q}")
                nc.sync.dma_start(out=w[:], in_=dram.ap()[q])
                return w
            b1 = None
            ones = None
            if has_b1 or has_bfc:
                ones = singles.tile([1, BC], mdt, tag="ones", name="ones")
                nc.vector.memset(ones[:], 1.0)
            if has_b1:
                b1 = [load_w(b1_d, 1, q, "b1") for q in range(4)]
            bfc = None
            if has_bfc:
                bfc = singles.tile([1, 1], mdt, tag="bfc", name="bfc")
                nc.sync.dma_start(out=bfc[:], in_=bfc_d.ap())

            # half-scale cell state: cs holds c/2 (fp16 so DVE 2x applies)
            cs = []
            for layer in range(2):
                c = singles.tile([H, BC], mdt, tag=f"c{layer}", name=f"c{layer}")
                nc.vector.memset(c[:], 0.0)
                cs.append(c)
            RING = 4
            rings = [
                singles.tile([H, RING * BC], mdt, tag=f"h{layer}", name=f"h{layer}")
                for layer in range(2)
            ]

            def hslot(layer, t):
                s = t % RING
                return rings[layer][:, s * BC : (s + 1) * BC]

            # PSUM layout: one full 2KB bank per (layer, gate); only the first
            # BC columns are used. No step rotation: step t+1's x-projection
            # (start=True, lazily zeroing the bank) is WAR-serialized after
            # sigma(t)'s read of the same bytes, so each bank's accumulation
            # groups stay strictly sequential: x(start) -> h(stop) -> sigma
            # read -> x(start) -> ...
            # Bank order: L0 i,f,g = 0-2; L1 i,f,g = 3-5; o-gates last (L0=6,
            # L1=7). Sigmoid over each layer's i,f,g fires after only 3 of 4
            # h-proj matmuls; the o-gate sigmoids run off the critical path.
            psu = psp.tile([H, 8, 512], f32, tag="ps", name="ps")

            def gate_out(layer, q):
                bank = layer * 3 + q if q < 3 else 6 + layer
                return psu[:, bank : bank + 1, 0:BC].rearrange("p a b -> p (a b)")

            def x_rhs(t):
                j, r = t // XCHUNK, t % XCHUNK
                return xts[j][:, r * BC : (r + 1) * BC]

            def emit_l0_x(t):
                for q in range(4):
                    nc.tensor.matmul(
                        gate_out(0, q), w0x[q], x_rhs(t),
                        start=True, stop=(t == 0),
                    )

            def emit_l0_h(t):
                for q in range(4):
                    nc.tensor.matmul(
                        gate_out(0, q), w0h[q], hslot(0, t - 1),
                        start=False, stop=True,
                    )

            def emit_l1_x(t):
                for q in range(4):
                    nc.tensor.matmul(
                        gate_out(1, q), w1x[q], hslot(0, t),
                        start=True, stop=(t == 0 and not has_b1),
                    )
                    if has_b1:
                        nc.tensor.matmul(
                            gate_out(1, q), b1[q][:], ones[:],
                            start=False, stop=(t == 0),
                        )

            def emit_l1_h(t):
                for q in range(4):
                    nc.tensor.matmul(
                        gate_out(1, q), w1h[q], hslot(1, t - 1),
                        start=False, stop=True,
                    )

            def emit_sig_o(layers):
                # sigmoid over o-gate bank(s); off the critical path (only
                # h' needs it, after tanh)
                sgo = sgp.tile([H, 2 * BC], mdt, tag="sgo", name="sgo")
                lo, hi = 6 + min(layers), 7 + max(layers)
                nc.scalar.activation(
                    sgo[:, (lo - 6) * BC : (hi - 6) * BC].rearrange(
                        "p (g b) -> p g b", g=hi - lo
                    ),
                    psu[:, lo:hi, 0:BC], Act.Sigmoid,
                )
                return sgo

            def act_dve(layer, t):
                # gate order in banks/sg: (i, f, g, o). Sigmoid over i,f,g
                # fires after only 3 of 4 h-proj matmuls; the o-gate sigmoid
                # is needed only by h' (after tanh), so it runs off-path.
                sg = sgp.tile([H, 3 * BC], mdt, tag=f"sg{layer}", name=f"sg{layer}")
                nc.scalar.activation(
                    sg[:].rearrange("p (g b) -> p g b", g=3),
                    psu[:, layer * 3 : layer * 3 + 3, 0:BC], Act.Sigmoid,
                )
                t1_ = tmpp.tile([H, BC], mdt, tag=f"t1_{layer}", name=f"t1_{layer}")
                # (sig_g - 0.5) * sig_i  == 0.5 * i * tanh(g_pre)
                nc.vector.scalar_tensor_tensor(
                    t1_[:], sg[:, 2 * BC : 3 * BC], 0.5, sg[:, 0:BC],
                    Alu.subtract, Alu.mult,
                )
                t2_ = tmpp.tile([H, BC], mdt, tag=f"t2_{layer}", name=f"t2_{layer}")
                nc.vector.tensor_mul(t2_[:], sg[:, BC : 2 * BC], cs[layer][:])
                # c' = t1 + t2 = (i*tanh(g_pre) + f*c)/2  (half-scale state)
                nc.vector.tensor_add(cs[layer][:], t1_[:], t2_[:])
                sgo = emit_sig_o((layer,))
                th = tmpp.tile([H, BC], mdt, tag=f"th{layer}", name=f"th{layer}")
                # tanh(c) = tanh(2*c')
                nc.scalar.activation(th[:], cs[layer][:], Act.Tanh, scale=2.0)
                nc.vector.tensor_mul(
                    hslot(layer, t), sgo[:, layer * BC : (layer + 1) * BC], th[:]
                )

            emit_l0_x(0)
            for t in range(nsteps):
                if t >= 1:
                    emit_l0_h(t)
                    emit_l1_x(t - 1)
                    if t - 1 >= 1:
                        emit_l1_h(t - 1)
                act_dve(0, t)
                if t >= 1:
                    act_dve(1, t - 1)
                # next step's x-projections, emitted after sigma(t)'s read so
                # the bank WAR serializes them behind it (off critical path)
                if t + 1 < nsteps:
                    emit_l0_x(t + 1)
            emit_l1_x(nsteps - 1)
            if nsteps - 1 >= 1:
                emit_l1_h(nsteps - 1)
            act_dve(1, nsteps - 1)

            # FC head reuses a closed L0 psum bank region.
            pf = psu[0:1, 0:1, 0:BC].rearrange("p a b -> p (a b)")
            nc.tensor.matmul(
                pf, wfc[:], hslot(1, nsteps - 1),
                start=True, stop=not has_bfc,
            )
            if has_bfc:
                nc.tensor.matmul(pf, bfc[:], ones[:], start=False, stop=True)
            ot = singles.tile([1, BC], f32, tag="ot", name="ot")
            nc.vector.tensor_copy(ot[:], pf)
            nc.sync.dma_start(out=out_d.ap(), in_=ot[:])

    nc.compile()
    return nc


def _prep_weights(Wih, Whh, b, in_dim, fold_bias):
    """Repack [4H, in] PyTorch-gate-order (i,f,g,o) weights into per-gate
    lhsT tiles [in(+1), H], g scaled by 2 (tanh(g) = 2*sigmoid(2g) - 1)."""
    order = [0, 1, 2, 3]  # i, f, g, o
    pad = 2 if fold_bias else 0
    wx = np.zeros((4, in_dim + pad, H), np.float32)
    wh = np.empty((4, H, H), np.float32)
    for qi, q in enumerate(order):
        scale = 2.0 if q == 2 else 1.0
        wx[qi, :in_dim] = (Wih[q * H : (q + 1) * H] * scale).T
        if fold_bias:
            wx[qi, in_dim] = b[q * H : (q + 1) * H] * scale
        wh[qi] = (Whh[q * H : (q + 1) * H] * scale).T
    return wx, wh


def kernel(x, Wih0, Whh0, b0, Wih1, Whh1, b1, Wfc, bfc, _nsteps=T):
    from concourse.bass_utils import run_bass_kernel_spmd

    x = np.asarray(x, np.float32)
    nsteps = _nsteps
    has_b1 = bool(np.any(np.asarray(b1)))
    has_bfc = bool(np.any(np.asarray(bfc)))

    w0x, w0h = _prep_weights(np.asarray(Wih0, np.float32), np.asarray(Whh0, np.float32),
                             np.asarray(b0, np.float32), I, True)
    w1x, w1h = _prep_weights(np.asarray(Wih1, np.float32), np.asarray(Whh1, np.float32),
                             np.asarray(b1, np.float32), H, False)
    # [4, K, H] -> [K, 4H] so each weight tensor lands in SBUF as one DMA
    w0x, w0h, w1x, w1h = (
        np.ascontiguousarray(w.transpose(1, 0, 2).reshape(w.shape[1], 4 * H))
        for w in (w0x, w0h, w1x, w1h)
    )
    wfc = np.ascontiguousarray(np.asarray(Wfc, np.float32).reshape(1, H).T)

    key = (has_b1, has_bfc, nsteps)
    if key not in _cache:
        _cache[key] = _build(has_b1, has_bfc, nsteps)
    nc = _cache[key]

    mnp = _mm_np_dtype()
    in_maps = []
    for c in range(NCORES):
        xc = x[c * BC : (c + 1) * BC, :nsteps]  # [BC, t, I]
        xt = np.zeros((I + 2, nsteps, BC), np.float32)
        xt[:I] = xc.transpose(2, 1, 0)
        xt[I] = 1.0
        m = {"xt": xt.astype(mnp), "w0x": w0x.astype(mnp), "w0h": w0h.astype(mnp),
             "w1x": w1x.astype(mnp), "w1h": w1h.astype(mnp), "wfc": wfc.astype(mnp)}
        if has_b1:
            border = [0, 1, 2, 3]
            bb = np.empty((4, 1, H), np.float32)
            for qi, q in enumerate(border):
                bb[qi, 0] = np.asarray(b1, np.float32)[q * H : (q + 1) * H] * (2.0 if q == 2 else 1.0)
            m["b1"] = bb.astype(mnp)
        if has_bfc:
            m["bfc"] = np.asarray(bfc, np.float32).reshape(1, 1).astype(mnp)
        in_maps.append(m)

    res = run_bass_kernel_spmd(nc, in_maps, list(range(NCORES)))
    globals()["LAST_RESULT"] = res
    globals()["LAST_RUN"] = (nc, in_maps)
    out = np.empty((B, 1), np.float32)
    for c in range(NCORES):
        out[c * BC : (c + 1) * BC, 0] = res.results[c]["out"][0]
    return out


def bench(iters=6):
    """Re-run the last compiled kernel, returning per-call wall seconds."""
    import time
    from concourse.bass_utils import run_bass_kernel_spmd

    nc, in_maps = globals()["LAST_RUN"]
    times = []
    for _ in range(iters):
        t0 = time.perf_counter()
        run_bass_kernel_spmd(nc, in_maps, list(range(NCORES)))
        times.append(time.perf_counter() - t0)
    return times

